# revision 25
# baseline (speedup 1.0000x reference)
"""GraphGym GeneralConv on 8 TRN2 cores — v4 edge-expanded streaming kernel.

Key idea (matmul associativity): with xr = (dis * x) expanded per-edge on
the host into destination-block-grouped layout, the per-block output is

    out_b = dis_d * ( (sum_t sel_t^T @ xr_t) @ W )

so the device never gathers: it streams xr sequentially (plain DMA, no
SWDGE descriptors), builds 0/1 selection matrices on DVE/Pool from the
per-slot destination columns, accumulates A_b = sel^T @ xr in PSUM via
chained matmuls (fp16, 64-wide moving = 1 cycle/row), then applies W once
per block (PE transpose + tiny matmul) and scales by dis_dest.

Every edge (including self-loops and parallel duplicates) gets a slot, so
the result is exact up to fp16 rounding — no host-side correction term.

Slot layout per core: block b owns tiles [T0S[b], T0S[b]+TBS[b]); slot
(p, t) of block b holds edge index base_b + p*TBS[b] + t.  TBS is the
max over cores so all 8 cores compile one SPMD program.
"""

import contextlib
import math

import numpy as np

N_NODES = 100000
DIM = 64
N_CORES = 8
P = 128


class Cfg:
    def __init__(self, n_nodes, dim, n_cores, grp=4, xbufs=6, selbufs=4,
                 psbufs=2, variant="B"):
        self.variant = variant
        self.N = n_nodes
        self.DIM = dim
        self.NC = n_cores
        self.NBLK = math.ceil(n_nodes / (n_cores * P))
        self.SHARD = self.NBLK * P
        self.grp = grp
        self.xbufs = xbufs
        self.selbufs = selbufs
        self.psbufs = psbufs
        # data-dependent structure, set by host_prep
        self.TBS = None
        self.T0S = None
        self.SUMT = None
        self.TMAX = None

    def set_structure(self, t2):
        # t2: [NBLK, 2] tiles per (block, col-half)
        self.T2 = [(int(a), int(b)) for a, b in t2]
        self.TBS = [a + b for a, b in self.T2]
        self.T0S = [0]
        for t in self.TBS:
            self.T0S.append(self.T0S[-1] + t)
        self.SUMT = self.T0S[-1]
        self.TMAX = max(self.TBS)
        self.W = 64  # sel width = one col-half

    def skey(self):
        return tuple(self.T2) if self.TBS else None


CFG = Cfg(N_NODES, DIM, N_CORES)


def host_prep(cfg, x, weight, edge_index):
    x = np.asarray(x, dtype=np.float32)
    weight = np.asarray(weight, dtype=np.float32)
    ei = np.asarray(edge_index)
    erow = ei[0].astype(np.int64)
    ecol = ei[1].astype(np.int64)

    deg = (np.bincount(erow, minlength=cfg.N) + 1).astype(np.float32)
    dis = deg ** -0.5
    xd = (x * dis[:, None]).astype(np.float16)

    loops = np.arange(cfg.N, dtype=np.int64)
    row = np.concatenate([erow, loops])
    col = np.concatenate([ecol, loops])

    k = np.minimum(col // cfg.SHARD, cfg.NC - 1)

    # per-(core, block, col-half) counts -> uniform structure (max over cores)
    blk = (col - k * cfg.SHARD) // P
    colloc = (col - k * cfg.SHARD) % P
    half = (colloc >= 64).astype(np.int64)
    cnt = np.zeros((cfg.NC, cfg.NBLK * 2), dtype=np.int64)
    np.add.at(cnt, (k, blk * 2 + half), 1)
    t2 = -(-cnt.max(axis=0) // P).reshape(cfg.NBLK, 2)
    cfg.set_structure(t2)
    t0s = np.asarray(cfg.T0S[:-1])
    tlo = np.asarray([a for a, _ in cfg.T2])
    SUMT = cfg.SUMT

    id128 = np.eye(P, dtype=np.float16)
    w16 = weight.astype(np.float16)
    iota_w = np.broadcast_to(np.arange(64, dtype=np.float16),
                             (P, 64)).copy()

    in_maps = []
    for core in range(cfg.NC):
        m = k == core
        u = row[m]
        bb = blk[m]
        cc = colloc[m]
        hh = half[m]
        g = bb * 2 + hh
        order = np.argsort(g, kind="stable")
        u, bb, cc, hh, g = u[order], bb[order], cc[order], hh[order], g[order]
        starts = np.concatenate([[0], np.cumsum(np.bincount(
            g, minlength=cfg.NBLK * 2))])
        pos = np.arange(g.size) - starts[g]
        pp = pos % P
        tau = t0s[bb] + np.where(hh == 1, tlo[bb], 0) + pos // P

        xr = np.zeros((P, SUMT, cfg.DIM), dtype=np.float16)
        colv = np.full((P, SUMT), -1.0, dtype=np.float16)
        xr[pp, tau] = xd[u]
        colv[pp, tau] = (cc - 64 * hh).astype(np.float16)

        nd = min(cfg.SHARD, cfg.N - core * cfg.SHARD)
        dd = np.zeros(cfg.SHARD, dtype=np.float32)
        dd[:nd] = dis[core * cfg.SHARD:core * cfg.SHARD + nd]
        dd = np.ascontiguousarray(
            dd.reshape(cfg.NBLK, P).T)  # [P, NBLK]

        in_maps.append({
            "xr": np.ascontiguousarray(xr.reshape(P, SUMT * cfg.DIM)),
            "colv": colv,
            "disd": dd,
            "iota": iota_w,
            "id128": id128,
            "w": w16,
        })
    return in_maps, None


def unshard(cfg, outs, corr):
    out = np.empty((cfg.N, cfg.DIM), dtype=np.float32)
    for core in range(cfg.NC):
        o = outs[core]["outp"].reshape(P, cfg.NBLK, cfg.DIM)
        o = o.transpose(1, 0, 2).reshape(cfg.NBLK * P, cfg.DIM)
        nd = min(cfg.SHARD, cfg.N - core * cfg.SHARD)
        out[core * cfg.SHARD:core * cfg.SHARD + nd] = o[:nd]
    if corr is not None:
        out += corr
    return out


_PROG_CACHE = {}


def build_program(cfg, reps=1, phases="2"):
    """phases: "2" full; "X" xr DMA only; "D" +sel builds; "M" sel+matmul
    with constant xr (no stream DMA)."""
    import concourse.tile as tile
    from concourse import bacc, mybir

    f16 = mybir.dt.float16
    f32 = mybir.dt.float32
    nc = bacc.Bacc("TRN2", target_bir_lowering=False, debug=False,
                   num_devices=cfg.NC)

    SUMT, TMAX, NBLK, W = cfg.SUMT, cfg.TMAX, cfg.NBLK, cfg.W
    xr = nc.dram_tensor("xr", [P, SUMT * cfg.DIM], f16, kind="ExternalInput")
    colv = nc.dram_tensor("colv", [P, SUMT], f16, kind="ExternalInput")
    disd = nc.dram_tensor("disd", [P, NBLK], f32, kind="ExternalInput")
    iota = nc.dram_tensor("iota", [P, W], f16, kind="ExternalInput")
    id128 = nc.dram_tensor("id128", [P, P], f16, kind="ExternalInput")
    w = nc.dram_tensor("w", [cfg.DIM, cfg.DIM], f16, kind="ExternalInput")
    outp = nc.dram_tensor("outp", [P, NBLK * cfg.DIM], f32,
                          kind="ExternalOutput")

    full = "2" in phases
    do_dma = full or "X" in phases or "D" in phases
    do_sel = full or "D" in phases or "M" in phases or "E" in phases
    do_mm = full or "M" in phases or "P" in phases
    const_sel = (not do_sel) and "P" in phases

    with tile.TileContext(nc) as tc:
      with (tc.For_i(0, reps, 1) if reps > 1 else contextlib.nullcontext()):
        with tc.tile_pool(name="c", bufs=1) as cp, \
             tc.tile_pool(name="xp", bufs=cfg.xbufs) as xp, \
             tc.tile_pool(name="sp", bufs=cfg.selbufs) as sp, \
             tc.tile_pool(name="ap", bufs=4) as ap, \
             tc.tile_pool(name="ppB", bufs=cfg.psbufs, space="PSUM") as ppB, \
             tc.tile_pool(name="ppT", bufs=2, space="PSUM") as ppT, \
             tc.tile_pool(name="ppO", bufs=2, space="PSUM") as ppO:
            w_sb = cp.tile([cfg.DIM, cfg.DIM], f16)
            nc.sync.dma_start(out=w_sb[:], in_=w[:])
            id_sb = cp.tile([P, P], f16)
            nc.sync.dma_start(out=id_sb[:], in_=id128[:])
            iota_sb = cp.tile([P, W], f16)
            nc.sync.dma_start(out=iota_sb[:], in_=iota[:])
            colv_sb = cp.tile([P, SUMT], f16)
            nc.sync.dma_start(out=colv_sb[:], in_=colv[:])
            disd_sb = cp.tile([P, NBLK], f32)
            nc.sync.dma_start(out=disd_sb[:], in_=disd[:])
            out_sb = cp.tile([P, NBLK * cfg.DIM], f32)

            if do_mm and not do_dma:
                xconst = cp.tile([P, TMAX, cfg.DIM], f16)
                nc.vector.memset(xconst[:], 0.25)
            if const_sel:
                selconst = cp.tile([P, TMAX, W], f16)
                nc.vector.memset(selconst[:], 0.25)

            # group blocks into DMA batches of cfg.grp blocks
            groups = []
            b = 0
            while b < NBLK:
                g = list(range(b, min(b + cfg.grp, NBLK)))
                groups.append(g)
                b += cfg.grp

            dma_engs = [nc.sync, nc.scalar, nc.gpsimd]
            for gi, g in enumerate(groups):
                t_lo = cfg.T0S[g[0]]
                t_hi = cfg.T0S[g[-1] + 1]
                if do_dma:
                    xb = xp.tile([P, t_hi - t_lo, cfg.DIM], f16, tag="xb")
                    dma_engs[gi % len(dma_engs)].dma_start(
                        out=xb[:],
                        in_=xr[:, t_lo * cfg.DIM:t_hi * cfg.DIM])
                if not (do_sel or do_mm):
                    # consume every xb tile so the DMAs survive DCE
                    nc.vector.tensor_copy(
                        out=out_sb[:, (gi % NBLK) * cfg.DIM:
                                   (gi % NBLK + 1) * cfg.DIM],
                        in_=xb[:, 0, :])
                    continue
                for b in g:
                    T = cfg.TBS[b]
                    tau0 = cfg.T0S[b]
                    if do_sel:
                        selb = sp.tile([P, T, W], f16, tag="sel")
                        nc.vector.tensor_tensor(
                            out=selb[:],
                            in0=iota_sb[:].unsqueeze(1)
                                .broadcast_to((P, T, W)),
                            in1=colv_sb[:, tau0:tau0 + T]
                                .broadcast_to((P, T, W)),
                            op=mybir.AluOpType.is_equal)
                    if not do_mm:
                        # consume sel so the build survives DCE
                        nc.vector.tensor_copy(
                            out=out_sb[:, b * cfg.DIM:(b + 1) * cfg.DIM],
                            in_=selb[:, 0, :min(W, cfg.DIM)])
                        continue
                    if const_sel:
                        selb = selconst
                    xv = xb if do_dma else xconst
                    toff = (tau0 - t_lo) if do_dma else 0
                    TLO = cfg.T2[b][0]
                    a_sb = ap.tile([P, cfg.DIM], f16, tag="a")
                    for h, lo_t, n_t in ((0, 0, TLO), (1, TLO, T - TLO)):
                        asl = a_sb[h * 64:(h + 1) * 64, :]
                        if n_t == 0:
                            nc.scalar.memzero(asl)
                            continue
                        psB = ppB.tile([64, cfg.DIM], f32,
                                       tag=f"ps{h}")
                        for i in range(n_t):
                            t = lo_t + i
                            nc.tensor.matmul(
                                out=psB[:],
                                lhsT=selb[:, t, :],
                                rhs=xv[:, toff + t, :],
                                start=(i == 0),
                                stop=(i == n_t - 1),
                                skip_group_check=True)
                        nc.scalar.copy(out=asl, in_=psB[:])
                    psT = ppT.tile([cfg.DIM, P], f16)
                    nc.tensor.transpose(psT[:], a_sb[:], id_sb[:])
                    aT_sb = ap.tile([cfg.DIM, P], f16, tag="aT")
                    nc.scalar.copy(out=aT_sb[:], in_=psT[:])
                    psO = ppO.tile([P, cfg.DIM], f32)
                    nc.tensor.matmul(
                        out=psO[:], lhsT=aT_sb[:], rhs=w_sb[:],
                        start=True, stop=True)
                    nc.vector.tensor_scalar_mul(
                        out_sb[:, b * cfg.DIM:(b + 1) * cfg.DIM],
                        psO[:], disd_sb[:, b:b + 1])
            nc.sync.dma_start(out=outp[:], in_=out_sb[:])

    nc.compile()
    return nc


def get_program(cfg, reps=1, phases="2"):
    key = (cfg.skey(), cfg.grp, cfg.xbufs, cfg.selbufs, cfg.psbufs,
           reps, phases)
    if key not in _PROG_CACHE:
        _PROG_CACHE[key] = build_program(cfg, reps=reps, phases=phases)
    return _PROG_CACHE[key]


def kernel(x, weight, edge_index):
    from concourse.bass_utils import run_bass_kernel_spmd

    cfg = CFG
    in_maps, corr = host_prep(cfg, x, weight, edge_index)
    nc = get_program(cfg)
    res = run_bass_kernel_spmd(nc, in_maps, list(range(cfg.NC)))
    return unshard(cfg, res.results, corr)


def build_with_queues(cfg, reps=1, phases="2", rotate=False):
    """Compatibility shim for test.py's timing path."""
    return build_program(cfg, reps=reps, phases=phases)


# revision 28
# speedup vs baseline: 1.0461x; 1.0461x over previous
"""GraphGym GeneralConv on 8 TRN2 cores — v4 edge-expanded streaming kernel.

Key idea (matmul associativity): with xr = (dis * x) expanded per-edge on
the host into destination-block-grouped layout, the per-block output is

    out_b = dis_d * ( (sum_t sel_t^T @ xr_t) @ W )

so the device never gathers: it streams xr sequentially (plain DMA, no
SWDGE descriptors), builds 0/1 selection matrices on DVE/Pool from the
per-slot destination columns, accumulates A_b = sel^T @ xr in PSUM via
chained matmuls (fp16, 64-wide moving = 1 cycle/row), then applies W once
per block (PE transpose + tiny matmul) and scales by dis_dest.

Every edge (including self-loops and parallel duplicates) gets a slot, so
the result is exact up to fp16 rounding — no host-side correction term.

Slot layout per core: block b owns tiles [T0S[b], T0S[b]+TBS[b]); slot
(p, t) of block b holds edge index base_b + p*TBS[b] + t.  TBS is the
max over cores so all 8 cores compile one SPMD program.
"""

import contextlib
import math

import numpy as np

N_NODES = 100000
DIM = 64
N_CORES = 8
P = 128


class Cfg:
    def __init__(self, n_nodes, dim, n_cores, grp=4, xbufs=6, selbufs=4,
                 psbufs=3, variant="A"):
        self.variant = variant
        self.N = n_nodes
        self.DIM = dim
        self.NC = n_cores
        self.NBLK = math.ceil(n_nodes / (n_cores * P))
        self.SHARD = self.NBLK * P
        self.grp = grp
        self.xbufs = xbufs
        self.selbufs = selbufs
        self.psbufs = psbufs
        # data-dependent structure, set by host_prep
        self.TBS = None
        self.T0S = None
        self.SUMT = None
        self.TMAX = None

    def set_structure(self, t2):
        # t2: [NBLK, 2] tiles per (block, col-half)
        self.T2 = [(int(a), int(b)) for a, b in t2]
        self.TBS = [a + b for a, b in self.T2]
        self.T0S = [0]
        for t in self.TBS:
            self.T0S.append(self.T0S[-1] + t)
        self.SUMT = self.T0S[-1]
        self.TMAX = max(self.TBS)
        self.W = 64  # sel width = one col-half

    def skey(self):
        return tuple(self.T2) if self.TBS else None


CFG = Cfg(N_NODES, DIM, N_CORES)


def host_prep(cfg, x, weight, edge_index):
    x = np.asarray(x, dtype=np.float32)
    weight = np.asarray(weight, dtype=np.float32)
    ei = np.asarray(edge_index)
    erow = ei[0].astype(np.int64)
    ecol = ei[1].astype(np.int64)

    deg = (np.bincount(erow, minlength=cfg.N) + 1).astype(np.float32)
    dis = deg ** -0.5
    xd = (x * dis[:, None]).astype(np.float16)

    loops = np.arange(cfg.N, dtype=np.int64)
    row = np.concatenate([erow, loops])
    col = np.concatenate([ecol, loops])

    k = np.minimum(col // cfg.SHARD, cfg.NC - 1)

    # per-(core, block, col-half) counts -> uniform structure (max over cores)
    blk = (col - k * cfg.SHARD) // P
    colloc = (col - k * cfg.SHARD) % P
    half = (colloc >= 64).astype(np.int64)
    cnt = np.zeros((cfg.NC, cfg.NBLK * 2), dtype=np.int64)
    np.add.at(cnt, (k, blk * 2 + half), 1)
    t2 = -(-cnt.max(axis=0) // P).reshape(cfg.NBLK, 2)
    cfg.set_structure(t2)
    t0s = np.asarray(cfg.T0S[:-1])
    tlo = np.asarray([a for a, _ in cfg.T2])
    SUMT = cfg.SUMT

    w16 = weight.astype(np.float16)
    iota_w = np.broadcast_to(np.arange(64, dtype=np.float16),
                             (P, 64)).copy()

    in_maps = []
    for core in range(cfg.NC):
        m = k == core
        u = row[m]
        bb = blk[m]
        cc = colloc[m]
        hh = half[m]
        g = bb * 2 + hh
        order = np.argsort(g, kind="stable")
        u, bb, cc, hh, g = u[order], bb[order], cc[order], hh[order], g[order]
        starts = np.concatenate([[0], np.cumsum(np.bincount(
            g, minlength=cfg.NBLK * 2))])
        pos = np.arange(g.size) - starts[g]
        pp = pos % P
        tau = t0s[bb] + np.where(hh == 1, tlo[bb], 0) + pos // P

        xr = np.zeros((P, SUMT, cfg.DIM), dtype=np.float16)
        colv = np.full((P, SUMT), -1.0, dtype=np.float16)
        xr[pp, tau] = xd[u]
        colv[pp, tau] = (cc - 64 * hh).astype(np.float16)

        nd = min(cfg.SHARD, cfg.N - core * cfg.SHARD)
        dd = np.zeros(cfg.SHARD, dtype=np.float32)
        dd[:nd] = dis[core * cfg.SHARD:core * cfg.SHARD + nd]
        dd = np.ascontiguousarray(
            dd.reshape(cfg.NBLK, P).T)  # [P, NBLK]

        in_maps.append({
            "xr": np.ascontiguousarray(xr.reshape(P, SUMT * cfg.DIM)),
            "colv": colv,
            "disd": dd,
            "iota": iota_w,
            "w": w16,
        })
    return in_maps, None


def unshard(cfg, outs, corr):
    out = np.empty((cfg.N, cfg.DIM), dtype=np.float32)
    for core in range(cfg.NC):
        o = outs[core]["outp"].reshape(P, cfg.NBLK, cfg.DIM)
        o = o.transpose(1, 0, 2).reshape(cfg.NBLK * P, cfg.DIM)
        nd = min(cfg.SHARD, cfg.N - core * cfg.SHARD)
        out[core * cfg.SHARD:core * cfg.SHARD + nd] = o[:nd]
    if corr is not None:
        out += corr
    return out


_PROG_CACHE = {}


def build_program(cfg, reps=1, phases="2"):
    """phases: "2" full; "X" xr DMA only; "D" +sel builds; "M" sel+matmul
    with constant xr (no stream DMA)."""
    import concourse.tile as tile
    from concourse import bacc, mybir

    f16 = mybir.dt.float16
    f32 = mybir.dt.float32
    nc = bacc.Bacc("TRN2", target_bir_lowering=False, debug=False,
                   num_devices=cfg.NC)

    SUMT, TMAX, NBLK, W = cfg.SUMT, cfg.TMAX, cfg.NBLK, cfg.W
    xr = nc.dram_tensor("xr", [P, SUMT * cfg.DIM], f16, kind="ExternalInput")
    colv = nc.dram_tensor("colv", [P, SUMT], f16, kind="ExternalInput")
    disd = nc.dram_tensor("disd", [P, NBLK], f32, kind="ExternalInput")
    iota = nc.dram_tensor("iota", [P, W], f16, kind="ExternalInput")
    w = nc.dram_tensor("w", [cfg.DIM, cfg.DIM], f16, kind="ExternalInput")
    outp = nc.dram_tensor("outp", [P, NBLK * cfg.DIM], f32,
                          kind="ExternalOutput")

    full = "2" in phases
    do_dma = full or "X" in phases or "D" in phases
    do_sel = full or "D" in phases or "M" in phases or "E" in phases
    do_mm = full or "M" in phases or "P" in phases
    const_sel = (not do_sel) and "P" in phases

    with tile.TileContext(nc) as tc:
      with (tc.For_i(0, reps, 1) if reps > 1 else contextlib.nullcontext()):
        with tc.tile_pool(name="c", bufs=1) as cp, \
             tc.tile_pool(name="xp", bufs=cfg.xbufs) as xp, \
             tc.tile_pool(name="sp", bufs=cfg.selbufs) as sp, \
             tc.tile_pool(name="ap", bufs=4) as ap, \
             tc.tile_pool(name="ppB", bufs=cfg.psbufs, space="PSUM") as ppB, \
             tc.tile_pool(name="ppO", bufs=2, space="PSUM") as ppO:
            w_sb = cp.tile([cfg.DIM, cfg.DIM], f16)
            nc.sync.dma_start(out=w_sb[:], in_=w[:])
            iota_sb = cp.tile([P, W], f16)
            nc.sync.dma_start(out=iota_sb[:], in_=iota[:])
            colv_sb = cp.tile([P, SUMT], f16)
            nc.sync.dma_start(out=colv_sb[:], in_=colv[:])
            disd_sb = cp.tile([P, NBLK], f32)
            nc.sync.dma_start(out=disd_sb[:], in_=disd[:])
            out_sb = cp.tile([P, NBLK * cfg.DIM], f32)

            if do_mm and not do_dma:
                xconst = cp.tile([P, TMAX, cfg.DIM], f16)
                nc.vector.memset(xconst[:], 0.25)
            if const_sel:
                selconst = cp.tile([P, TMAX, W], f16)
                nc.vector.memset(selconst[:], 0.25)

            # group blocks into DMA batches of cfg.grp blocks
            groups = []
            b = 0
            while b < NBLK:
                g = list(range(b, min(b + cfg.grp, NBLK)))
                groups.append(g)
                b += cfg.grp

            dma_engs = [nc.sync, nc.scalar, nc.gpsimd]
            for gi, g in enumerate(groups):
                t_lo = cfg.T0S[g[0]]
                t_hi = cfg.T0S[g[-1] + 1]
                if do_dma:
                    xb = xp.tile([P, t_hi - t_lo, cfg.DIM], f16, tag="xb")
                    dma_engs[gi % len(dma_engs)].dma_start(
                        out=xb[:],
                        in_=xr[:, t_lo * cfg.DIM:t_hi * cfg.DIM])
                if not (do_sel or do_mm):
                    # consume every xb tile so the DMAs survive DCE
                    nc.vector.tensor_copy(
                        out=out_sb[:, (gi % NBLK) * cfg.DIM:
                                   (gi % NBLK + 1) * cfg.DIM],
                        in_=xb[:, 0, :])
                    continue
                for b in g:
                    T = cfg.TBS[b]
                    tau0 = cfg.T0S[b]
                    if do_sel:
                        selb = sp.tile([P, T, W], f16, tag="sel")
                        nc.vector.tensor_tensor(
                            out=selb[:],
                            in0=iota_sb[:].unsqueeze(1)
                                .broadcast_to((P, T, W)),
                            in1=colv_sb[:, tau0:tau0 + T]
                                .broadcast_to((P, T, W)),
                            op=mybir.AluOpType.is_equal)
                    if not do_mm:
                        # consume sel so the build survives DCE
                        nc.vector.tensor_copy(
                            out=out_sb[:, b * cfg.DIM:(b + 1) * cfg.DIM],
                            in_=selb[:, 0, :min(W, cfg.DIM)])
                        continue
                    if const_sel:
                        selb = selconst
                    xv = xb if do_dma else xconst
                    toff = (tau0 - t_lo) if do_dma else 0
                    TLO = cfg.T2[b][0]
                    aT_sb = ap.tile([cfg.DIM, P], f16, tag="aT")
                    for h, lo_t, n_t in ((0, 0, TLO), (1, TLO, T - TLO)):
                        asl = aT_sb[:, h * 64:(h + 1) * 64]
                        if n_t == 0:
                            nc.scalar.memzero(asl)
                            continue
                        psA = ppB.tile([cfg.DIM, 64], f32,
                                       tag=f"ps{h}")
                        for i in range(n_t):
                            t = lo_t + i
                            nc.tensor.matmul(
                                out=psA[:],
                                lhsT=xv[:, toff + t, :],
                                rhs=selb[:, t, :],
                                start=(i == 0),
                                stop=(i == n_t - 1),
                                skip_group_check=True)
                        nc.scalar.copy(out=asl, in_=psA[:])
                    psO = ppO.tile([P, cfg.DIM], f32)
                    nc.tensor.matmul(
                        out=psO[:], lhsT=aT_sb[:], rhs=w_sb[:],
                        start=True, stop=True)
                    nc.scalar.mul(
                        out_sb[:, b * cfg.DIM:(b + 1) * cfg.DIM],
                        psO[:], disd_sb[:, b:b + 1])
            nc.sync.dma_start(out=outp[:], in_=out_sb[:])

    nc.compile()
    return nc


def get_program(cfg, reps=1, phases="2"):
    key = (cfg.skey(), cfg.grp, cfg.xbufs, cfg.selbufs, cfg.psbufs,
           reps, phases)
    if key not in _PROG_CACHE:
        _PROG_CACHE[key] = build_program(cfg, reps=reps, phases=phases)
    return _PROG_CACHE[key]


def kernel(x, weight, edge_index):
    from concourse.bass_utils import run_bass_kernel_spmd

    cfg = CFG
    in_maps, corr = host_prep(cfg, x, weight, edge_index)
    nc = get_program(cfg)
    res = run_bass_kernel_spmd(nc, in_maps, list(range(cfg.NC)))
    return unshard(cfg, res.results, corr)


def build_with_queues(cfg, reps=1, phases="2", rotate=False):
    """Compatibility shim for test.py's timing path."""
    return build_program(cfg, reps=reps, phases=phases)
